# revision 2
# baseline (speedup 1.0000x reference)
"""Trainium2 Bass kernel for nn_Attention_49074296324413 — v4.

Per-core (data-parallel over batch):
  kv = dw3x3(conv1x1(x, w_kv)); k, v = split(kv)
  k  = avgpool2x2(k) [folded 4x4-stride-2 depthwise]
  q  = conv3x3(conv1x1(y, w_q))
  attn = softmax(norm(q) @ norm(k).T * temp); out = w_proj @ (attn @ v)

v4 strategy:
  - fp8 DoubleRow matmuls wherever precision allows: q path (normalized),
    k path (normalized), and the v depthwise as VALUE+RESIDUAL pairs
    (v1 = v8 + r8 exactly compensates fp8 quantization to ~0.1%).
  - all depthwise convs are diagonal-weight DR matmuls on the tensor
    engine, 2 taps (or value+residual) per instruction; diagonals baked
    on host and DMA'd.
  - DMA: one issue per logical tensor via rearranged DRAM APs (partition
    dim from the row-block dim); critical tensors first per ring.
  - PSUM->SBUF copies split Act/DVE; output bf16.
"""
import numpy as np
import ml_dtypes

import concourse.bass as bass
import concourse.tile as tile
from concourse import bacc, mybir
from concourse.bass_utils import run_bass_kernel_spmd

dt = mybir.dt
BF = dt.bfloat16
F32 = dt.float32
FP8 = dt.float8e4
AF = mybir.ActivationFunctionType
OP = mybir.AluOpType
PM = mybir.MatmulPerfMode

DIM = 384
HEADS = 8
HC = DIM // HEADS
CT = DIM // 128
H = 64
NPIX = H * H
PW = H + 2                 # 66
H2 = 32
NPIX2 = H2 * H2

K_SCALE = 8.0              # on wkv8 k half (cancelled by k norm)
Q_SCALE = 50.0             # on wq8 / wqdw8 (cancelled by q norm)
W4K_SCALE = 256.0          # on k diag weights (cancelled by k norm)
W3V_SCALE = 64.0           # on v diag weights (compensated via wpT/64)

# D conv planes: p0=(0,0) p1=(0,1) p2=(0,2) p3=(1,2) shifts of q1pad.
D_GROUPS = [(0, 0), (0, 1), (0, 2), (2, 0)]  # (plane_base, dy) per DR group

BF_NP = ml_dtypes.bfloat16
F8_NP = ml_dtypes.float8_e4m3


def build_program():
    nc = bacc.Bacc("TRN2", target_bir_lowering=False, debug=False)

    xb_d = nc.dram_tensor("x_bf", (DIM, NPIX), BF, kind="ExternalInput")
    x8_d = nc.dram_tensor("x_f8", (DIM, NPIX), FP8, kind="ExternalInput")
    y8_d = nc.dram_tensor("y_f8", (DIM, NPIX2), FP8, kind="ExternalInput")
    wkvv_d = nc.dram_tensor("wkvv", (DIM, DIM), BF, kind="ExternalInput")
    wkv8_d = nc.dram_tensor("wkv8", (DIM, DIM), FP8, kind="ExternalInput")
    wq8_d = nc.dram_tensor("wq8", (DIM, DIM), FP8, kind="ExternalInput")
    wqdw8_d = nc.dram_tensor("wqdw8", (DIM, 5 * 3 * 2 * 128), FP8,
                             kind="ExternalInput")
    kdiag_d = nc.dram_tensor("kdiag8", (128, 3 * 8 * 2 * 128), FP8,
                             kind="ExternalInput")
    vdiag_d = nc.dram_tensor("vdiagb", (128, 3 * 9 * 128), BF,
                             kind="ExternalInput")
    wpT_d = nc.dram_tensor("wpT", (DIM, DIM), BF, kind="ExternalInput")
    temp_d = nc.dram_tensor("temp_col", (DIM, 1), F32, kind="ExternalInput")
    idn_d = nc.dram_tensor("idn", (128, 128), BF, kind="ExternalInput")

    out_d = nc.dram_tensor("out", (DIM, NPIX), BF, kind="ExternalOutput")

    with tile.TileContext(nc) as tc:
        _emit(nc, tc, xb_d, x8_d, y8_d, wkvv_d, wkv8_d, wq8_d, wqdw8_d,
              kdiag_d, vdiag_d, wpT_d, temp_d, idn_d, out_d)
    nc.compile()
    return nc


def _blk(d, p=128):
    """DRAM AP [C*p, N] -> [p, C, N] (partition dim from row blocks)."""
    return d.rearrange("(c p) n -> p c n", p=p)


def _emit(nc, tc, xb_d, x8_d, y8_d, wkvv_d, wkv8_d, wq8_d, wqdw8_d,
          kdiag_d, vdiag_d, wpT_d, temp_d, idn_d, out_d):
    from contextlib import ExitStack
    ctx = ExitStack()

    cst = ctx.enter_context(tc.tile_pool(name="cst", bufs=1))
    big = ctx.enter_context(tc.tile_pool(name="big", bufs=1))
    xvp = ctx.enter_context(tc.tile_pool(name="xvp", bufs=1))
    wrk = ctx.enter_context(tc.tile_pool(name="wrk", bufs=2))
    osb = ctx.enter_context(tc.tile_pool(name="osb", bufs=3))
    ps_big = ctx.enter_context(tc.tile_pool(name="ps_big", bufs=3, space="PSUM"))
    ps_t = ctx.enter_context(tc.tile_pool(name="ps_t", bufs=2, space="PSUM"))

    # ---------------- DMA issues (one per tensor, priority order) ----------
    # sync: y8, wq8, x8, idn, temp, kdiag, vdiag
    y8 = cst.tile([128, CT, NPIX2], FP8, tag="y8", name="y8")
    nc.sync.dma_start(y8[:], _blk(y8_d.ap()))
    wq8p = cst.tile([128, CT, DIM], FP8, tag="wq8", name="wq8")
    nc.sync.dma_start(wq8p[:], _blk(wq8_d.ap()))
    x8 = cst.tile([128, CT, NPIX], FP8, tag="x8", name="x8")
    nc.sync.dma_start(x8[:], _blk(x8_d.ap()))
    idn_t = cst.tile([128, 128], BF, tag="idn", name="idn")
    nc.sync.dma_start(idn_t[:], idn_d.ap())
    tempc = cst.tile([128, CT, 1], F32, tag="tempc", name="tempc")
    nc.sync.dma_start(tempc[:], _blk(temp_d.ap()))
    kdiag = cst.tile([128, CT, 8, 2, 128], FP8, tag="kdiag", name="kdiag")
    nc.sync.dma_start(kdiag[:].rearrange("p a b c d -> p (a b c d)"),
                      kdiag_d.ap())
    vdiag = cst.tile([128, CT, 9, 128], BF, tag="vdiag", name="vdiag")
    nc.sync.dma_start(vdiag[:].rearrange("p a b c -> p (a b c)"),
                      vdiag_d.ap())

    # scalar ring: wkvv first (A-v dep), xb g0/g1, wqdw8, wkv8, wpT
    wkvv = cst.tile([128, CT, DIM], BF, tag="wkvv", name="wkvv")
    nc.scalar.dma_start(wkvv[:], _blk(wkvv_d.ap()))
    xb = xvp.tile([128, CT, NPIX], BF, tag="xv", name="xb")
    for g in range(2):
        nc.scalar.dma_start(xb[:, :, 1024 * g:1024 * (g + 1)],
                            _blk(xb_d.ap()[:, 1024 * g:1024 * (g + 1)]))
    wqdw8 = cst.tile([128, CT, 5, 3, 2, 128], FP8, tag="wqdw8", name="wqdw8")
    nc.scalar.dma_start(wqdw8[:].rearrange("p a b c d e -> p a (b c d e)"),
                        _blk(wqdw8_d.ap()))
    wpT_t = []
    for h in range(HEADS):
        t = cst.tile([HC, DIM], BF, tag=f"wpT{h}", name=f"wpT{h}")
        nc.scalar.dma_start(t[:], wpT_d.ap()[HC * h:HC * (h + 1), :])
        wpT_t.append(t)

    # gpsimd ring: xb g2/g3, wkv8
    for g in range(2, 4):
        nc.gpsimd.dma_start(xb[:, :, 1024 * g:1024 * (g + 1)],
                            _blk(xb_d.ap()[:, 1024 * g:1024 * (g + 1)]))
    wkv8p = cst.tile([128, CT, DIM], FP8, tag="wkv8", name="wkv8")
    nc.gpsimd.dma_start(wkv8p[:], _blk(wkv8_d.ap()))

    # ---------------- padded buffers ----------------
    # vp8: [128, 2(value,residual), 66, 66] fp8
    # kpad8: [128, 2(plane0, rows+2), 66, 66] fp8
    # q1p8: [128, 4 shifted planes, 36, 32] fp8
    vp8, kpad8, q1p8 = [], [], []
    for ct in range(CT):
        t = big.tile([128, PW, PW], BF, tag=f"vp8{ct}")
        nc.gpsimd.memset(t[:, 0, :], 0.0)
        nc.gpsimd.memset(t[:, PW - 1, :], 0.0)
        nc.gpsimd.memset(t[:, 1:PW - 1, 0:1], 0.0)
        nc.gpsimd.memset(t[:, 1:PW - 1, PW - 1:PW], 0.0)
        vp8.append(t)
        t = big.tile([128, 2, PW, PW], FP8, tag=f"kpad8{ct}")
        nc.gpsimd.memset(t[:, 0, 0, :], 0.0)
        nc.gpsimd.memset(t[:, 0, PW - 1, :], 0.0)
        nc.gpsimd.memset(t[:, 0, 1:PW - 1, 0:1], 0.0)
        nc.gpsimd.memset(t[:, 0, 1:PW - 1, PW - 1:PW], 0.0)
        kpad8.append(t)
        t = big.tile([128, 4, 36, 32], FP8, tag=f"q1p8{ct}")
        nc.gpsimd.memset(t[:, 0, 0, :], 0.0)
        nc.gpsimd.memset(t[:, 0, 33:36, :], 0.0)
        nc.gpsimd.memset(t[:, 0, 1:33, 0:1], 0.0)
        nc.gpsimd.memset(t[:, 1, 0, :], 0.0)
        nc.gpsimd.memset(t[:, 1, 33:36, :], 0.0)
        nc.gpsimd.memset(t[:, 2, 0, :], 0.0)
        nc.gpsimd.memset(t[:, 2, 33:36, :], 0.0)
        nc.gpsimd.memset(t[:, 2, 1:33, 31:32], 0.0)
        nc.gpsimd.memset(t[:, 3, 32:36, :], 0.0)
        nc.gpsimd.memset(t[:, 3, 0:32, 31:32], 0.0)
        q1p8.append(t)

    eps_col = cst.tile([128, 1], F32, tag="eps_col", name="eps_col")
    nc.vector.memset(eps_col[:], 1e-24)
    zero_col = cst.tile([128, 1], F32, tag="zero_col", name="zero_col")
    nc.vector.memset(zero_col[:], 0.0)
    # fp8-output warmup (first fp8-dst op on each engine loads ucode ~9us)
    wu = cst.tile([128, 2], FP8, tag="wu", name="wu")
    wub = cst.tile([128, 2], BF, tag="wub", name="wub")
    nc.vector.memset(wub[:], 0.0)
    nc.vector.scalar_tensor_tensor(out=wu[:], in0=wub[:], scalar=1.0,
                                   in1=wub[:], op0=OP.mult, op1=OP.add)
    nc.vector.scalar_tensor_tensor(out=wu[:], in0=wub[:], scalar=1.0,
                                   in1=wub[:], op0=OP.mult, op1=OP.add)
    nc.scalar.activation(wu[:], wub[:], AF.Copy)
    nc.scalar.activation(wu[:], wub[:], AF.Copy)

    # ---------------- Phase C: q1 = W_q @ y -> q1p8 planes (fp8 DR) --------
    for co in range(CT):
        ps = ps_big.tile([128, 1024], F32, tag="ps", name="ps_c")
        for j in range(2):
            rhs2 = y8[:, 0:2, 512 * j:512 * (j + 1)]
            nc.tensor.matmul(ps[:, 512 * j:512 * (j + 1)],
                             wq8p[:, 0:2, 128 * co:128 * (co + 1)],
                             rhs2, start=True, stop=False,
                             perf_mode=PM.DoubleRow)
            nc.tensor.matmul(ps[:, 512 * j:512 * (j + 1)],
                             wq8p[:, 2, 128 * co:128 * (co + 1)],
                             y8[:, 2, 512 * j:512 * (j + 1)],
                             start=False, stop=True)
        pv = ps[:].rearrange("p (a b) -> p a b", a=32)
        nc.scalar.copy(q1p8[co][:, 0, 1:33, 1:32], pv[:, :, 0:31])
        nc.scalar.copy(q1p8[co][:, 1, 1:33, 0:32], pv)
        nc.scalar.copy(q1p8[co][:, 2, 1:33, 0:31], pv[:, :, 1:32])
        # plane3 = plane2 shifted up 1 row (DVE replicate)
        nc.vector.tensor_copy(q1p8[co][:, 3, 0:32, 0:31],
                              q1p8[co][:, 2, 1:33, 0:31])

    # ---------------- Phase A (v half, bf16) + v8/r8 build ----------------
    for g in range(4):
        for co in [3, 4, 5]:
            ct = co % 3
            ps = ps_big.tile([128, 1024], F32, tag="ps", name="ps_av")
            for ci in range(CT):
                for j in range(2):
                    nc.tensor.matmul(
                        ps[:, 512 * j:512 * (j + 1)],
                        wkvv[:, ci, 128 * ct:128 * (ct + 1)],
                        xb[:, ci, 1024 * g + 512 * j:1024 * g + 512 * (j + 1)],
                        start=(ci == 0), stop=(ci == CT - 1))
            r0 = 1 + 16 * g
            pv = ps[:].rearrange("p (a b) -> p a b", a=16)
            # value plane (Act) then residual plane (DVE, reads psum & v8)
            nc.scalar.copy(vp8[ct][:, r0:r0 + 16, 1:65], pv)

    # ---------------- Phase A (k half, fp8 DR, 2-pass LDW reuse) -----------
    for ct in range(CT):
        for gp in range(2):
            pss = [ps_big.tile([128, 1024], F32, tag="ps", name="ps_ak")
                   for _ in range(2)]
            for c4 in range(4):
                off = 2048 * gp + 512 * c4
                nc.tensor.matmul(pss[c4 // 2][:, 512 * (c4 % 2):512 * (c4 % 2 + 1)],
                                 wkv8p[:, 0:2, 128 * ct:128 * (ct + 1)],
                                 x8[:, 0:2, off:off + 512],
                                 start=True, stop=False, perf_mode=PM.DoubleRow)
            for c4 in range(4):
                off = 2048 * gp + 512 * c4
                nc.tensor.matmul(pss[c4 // 2][:, 512 * (c4 % 2):512 * (c4 % 2 + 1)],
                                 wkv8p[:, 2, 128 * ct:128 * (ct + 1)],
                                 x8[:, 2, off:off + 512],
                                 start=False, stop=True)
            for gg in range(2):
                r0 = 1 + 16 * (2 * gp + gg)
                nc.scalar.copy(kpad8[ct][:, 0, r0:r0 + 16, 1:65],
                               pss[gg][:].rearrange("p (a b) -> p a b", a=16))
        nc.vector.tensor_copy(kpad8[ct][:, 1, 0:64, :],
                              kpad8[ct][:, 0, 2:66, :])

    # ---------------- Phase D: q3 3x3 conv, fp8 DR ----------------
    q3 = [big.tile([128, NPIX2], BF, tag=f"q3n{ct}", name=f"q3n{ct}")
          for ct in range(CT)]
    kpT = [big.tile([128, DIM], BF, tag=f"kpT{pt}", name=f"kpT{pt}")
           for pt in range(8)]
    q3T = [big.tile([128, DIM], BF, tag=f"q3T{pt}", name=f"q3T{pt}")
           for pt in range(8)]
    kp16 = [big.tile([128, NPIX2], BF, tag=f"kp16{ct}", name=f"kp16{ct}")
            for ct in range(CT)]

    for co in range(CT):
        ps = ps_big.tile([128, 1024], F32, tag="ps", name="ps_q3")
        for ci in range(CT):
            first = (ci == 0)
            for gi, (pl0, dy) in enumerate(D_GROUPS):
                for j in range(2):
                    rhs = q1p8[ci][:, pl0:pl0 + 2,
                                   16 * j + dy:16 * j + dy + 16, :]
                    nc.tensor.matmul(
                        ps[:, 512 * j:512 * (j + 1)],
                        wqdw8[:, ci, gi, co, :, :], rhs,
                        start=(first and gi == 0), stop=False,
                        perf_mode=PM.DoubleRow)
            for j in range(2):
                nc.tensor.matmul(
                    ps[:, 512 * j:512 * (j + 1)],
                    wqdw8[:, ci, 4, co, 0, :],
                    q1p8[ci][:, 2, 16 * j + 2:16 * j + 18, :],
                    start=False, stop=(ci == CT - 1))
        nc.scalar.copy(q3[co][:], ps[:])
        nrm2 = wrk.tile([128, 1], F32, tag="nrm2q", name="nrm2q")
        sqq = wrk.tile([128, NPIX2], BF, tag="sqq", name="sqq")
        nc.scalar.activation(sqq[:], q3[co][:], AF.Square, bias=zero_col[:],
                             accum_out=nrm2[:])
        nrm = wrk.tile([128, 1], F32, tag="nrmq", name="nrmq")
        nc.scalar.activation(nrm[:], nrm2[:], AF.Sqrt, bias=eps_col[:])
        inv = wrk.tile([128, 1], F32, tag="invq", name="invq")
        nc.vector.reciprocal(inv[:], nrm[:])
        invt = wrk.tile([128, 1], F32, tag="invqt", name="invqt")
        nc.vector.tensor_mul(invt[:], inv[:], tempc[:, co, :])
        nc.vector.tensor_scalar_mul(q3[co][:], q3[co][:], invt[:])

    # ---------------- k depthwise+pool: fp8 DR diag ----------------
    for ct in range(CT):
        psk = ps_big.tile([128, 1024], F32, tag="ps", name="ps_k")
        kv = kpad8[ct][:].rearrange(
            "p pl (rp two) (cp ctwo) -> p pl rp two cp ctwo", two=2, ctwo=2)
        for g in range(8):
            uy, ux = g // 4, g % 4
            for h2 in range(2):
                ph = psk[:, 512 * h2:512 * (h2 + 1)].rearrange(
                    "p (a b) -> p a b", a=16)
                rhs = kv[:, 0:2, 16 * h2:16 * h2 + 16, uy,
                         ux // 2:ux // 2 + 32, ux % 2]
                nc.tensor.matmul(ph, kdiag[:, ct, g, :, :], rhs,
                                 start=(g == 0), stop=(g == 7),
                                 perf_mode=PM.DoubleRow)
        nrm2 = wrk.tile([128, 1], F32, tag="nrm2k", name="nrm2k")
        sqk = wrk.tile([128, NPIX2], BF, tag="sqk", name="sqk")
        nc.scalar.activation(sqk[:], psk[:], AF.Square, bias=zero_col[:],
                             accum_out=nrm2[:])
        nrm = wrk.tile([128, 1], F32, tag="nrmk", name="nrmk")
        nc.scalar.activation(nrm[:], nrm2[:], AF.Sqrt, bias=eps_col[:])
        inv = wrk.tile([128, 1], F32, tag="invk", name="invk")
        nc.vector.reciprocal(inv[:], nrm[:])
        nc.vector.tensor_scalar_mul(kp16[ct][:], psk[:], inv[:])

    # ---------------- v depthwise: value+residual fp8 DR diag --------------
    # reuses xb's SBUF (xv tag, bufs=1): all xb readers finish in phase A-v
    vdw_all = xvp.tile([128, CT, NPIX], BF, tag="xv", name="vdw")
    v_dw = [vdw_all[:, ct, :] for ct in range(CT)]
    for ct in range(CT):
        for qp in range(2):
            pss = [ps_big.tile([128, 1024], F32, tag="ps", name="ps_v")
                   for _ in range(2)]
            for t9 in range(9):
                dy, dx = t9 // 3, t9 % 3
                for c4 in range(4):
                    pj = pss[c4 // 2][:, 512 * (c4 % 2):512 * (c4 % 2 + 1)]
                    r0 = 32 * qp + 8 * c4 + dy
                    rhs = vp8[ct][:, r0:r0 + 8, dx:dx + H]
                    nc.tensor.matmul(pj.rearrange("p (a b) -> p a b", a=8),
                                     vdiag[:, ct, t9, :], rhs,
                                     start=(t9 == 0), stop=(t9 == 8))
            for gg in range(2):
                q = 2 * qp + gg
                if gg == 0:
                    nc.scalar.copy(v_dw[ct][:, 1024 * q:1024 * (q + 1)],
                                   pss[gg][:])
                else:
                    nc.vector.tensor_copy(v_dw[ct][:, 1024 * q:1024 * (q + 1)],
                                          pss[gg][:])

    # ---------------- B4/E2: kpT, q3T via PE transpose ----------------
    for ct in range(CT):
        for pt in range(8):
            pst = ps_t.tile([128, 128], BF, tag="ps_t", name="ps_tr")
            nc.tensor.transpose(pst[:], kp16[ct][:, 128 * pt:128 * (pt + 1)],
                                idn_t[:])
            nc.vector.tensor_copy(kpT[pt][:, 128 * ct:128 * (ct + 1)], pst[:])
    for ct in range(CT):
        for pt in range(8):
            pst = ps_t.tile([128, 128], BF, tag="ps_t", name="ps_trq")
            nc.tensor.transpose(pst[:], q3[ct][:, 128 * pt:128 * (pt + 1)],
                                idn_t[:])
            nc.vector.tensor_copy(q3T[pt][:, 128 * ct:128 * (ct + 1)], pst[:])

    # ---------------- F: QK + softmax + M (per head) ----------------
    mst = [big.tile([128, DIM], BF, tag=f"mst{ct}", name=f"mst{ct}")
           for ct in range(CT)]
    att_n = []
    for h in range(HEADS):
        cs = slice(HC * h, HC * (h + 1))
        pa = ps_t.tile([HC, HC], F32, tag="ps_t", name="ps_at")
        for pt in range(8):
            nc.tensor.matmul(pa[:], q3T[pt][:, cs], kpT[pt][:, cs],
                             start=(pt == 0), stop=(pt == 7))
        ae = wrk.tile([HC, HC], BF, tag=f"ae{h % 2}", name=f"ae{h % 2}", bufs=2)
        nc.scalar.activation(ae[:], pa[:], AF.Exp, bias=zero_col[0:HC, :])
        zs = wrk.tile([HC, 1], F32, tag="zs", name="zs")
        nc.vector.tensor_reduce(zs[:], ae[:], axis=mybir.AxisListType.X,
                                op=OP.add)
        zi = wrk.tile([HC, 1], F32, tag="zi", name="zi")
        nc.vector.reciprocal(zi[:], zs[:])
        an = wrk.tile([HC, HC], BF, tag=f"an{h}", name=f"an{h}")
        nc.vector.tensor_scalar_mul(an[:], ae[:], zi[:])
        att_n.append(an)
    for h in range(HEADS):
        pm = ps_t.tile([HC, DIM], F32, tag="ps_t", name="ps_M")
        nc.tensor.matmul(pm[:], att_n[h][:], wpT_t[h][:], start=True, stop=True)
        stg = wrk.tile([HC, DIM], BF, tag=f"stg{h % 2}", name=f"stg{h % 2}")
        nc.vector.tensor_copy(stg[:], pm[:])
        g0 = HC * h
        t0, o0 = divmod(g0, 128)
        n0 = min(128 - o0, HC)
        nc.sync.dma_start(mst[t0][o0:o0 + n0, :], stg[0:n0, :])
        if n0 < HC:
            nc.sync.dma_start(mst[t0 + 1][0:HC - n0, :], stg[n0:HC, :])

    # ---------------- H: out = Mst.T @ v_dw ----------------
    for ob in range(CT):
        for g4 in range(4):
            ps = ps_big.tile([128, 1024], F32, tag="ps", name="ps_h")
            for ctd in range(CT):
                for j in range(2):
                    nc.tensor.matmul(
                        ps[:, 512 * j:512 * (j + 1)],
                        mst[ctd][:, 128 * ob:128 * (ob + 1)],
                        v_dw[ctd][:, 1024 * g4 + 512 * j:1024 * g4 + 512 * (j + 1)],
                        start=(ctd == 0), stop=(ctd == CT - 1))
            ot = osb.tile([128, 1024], BF, tag="osb", name="osb", bufs=3)
            if g4 % 2 == 0:
                nc.scalar.copy(ot[:], ps[:])
            else:
                nc.vector.tensor_copy(ot[:], ps[:])
            eng = nc.sync if g4 % 2 == 0 else nc.scalar
            eng.dma_start(out_d.ap()[128 * ob:128 * (ob + 1),
                                     1024 * g4:1024 * (g4 + 1)], ot[:])
    ctx.close()


# ======================= host-side wrapper =======================

def _prep_shared(w_kv, w_kv_dw, w_q, w_q_dw, w_proj, temperature):
    w_kv = np.asarray(w_kv, np.float32)[:, :, 0, 0]          # [768, 384]
    w_kv_dw = np.asarray(w_kv_dw, np.float32)[:, 0]          # [768, 3, 3]
    w_q = np.asarray(w_q, np.float32)[:, :, 0, 0]            # [384, 384]
    w_q_dw = np.asarray(w_q_dw, np.float32)                  # [384, 384, 3, 3]
    w_proj = np.asarray(w_proj, np.float32)[:, :, 0, 0]      # [384, 384]
    temperature = np.asarray(temperature, np.float32).reshape(HEADS)

    # wkvv: v-half 1x1 weights, [in 384, out 384] transposed, bf16
    wkvv = np.ascontiguousarray(w_kv[DIM:].T).astype(BF_NP)
    # wkv8: k-half fp8 [in 384, ci-major 3 x out 384]; scale K_SCALE
    wk = w_kv[:DIM].T * K_SCALE                              # [in, out]
    wkv8 = np.ascontiguousarray(wk).astype(F8_NP)            # rows = in
    # wq8 similarly, scaled Q_SCALE
    wq8 = np.ascontiguousarray(w_q.T * Q_SCALE).astype(F8_NP)

    w3v = w_kv_dw[DIM:].reshape(DIM, 9)
    w3k = w_kv_dw[:DIM]
    w4k = np.zeros((DIM, 4, 4), np.float32)
    for uy in range(4):
        for ux in range(4):
            acc = np.zeros(DIM, np.float32)
            for dy in range(2):
                for dx in range(2):
                    ky, kx = uy - dy, ux - dx
                    if 0 <= ky < 3 and 0 <= kx < 3:
                        acc += w3k[:, ky, kx]
            w4k[:, uy, ux] = 0.25 * acc * W4K_SCALE
    w4k = w4k.reshape(DIM, 16)

    # diag weights baked on host
    ey = np.eye(128, dtype=np.float32)
    kdiag = np.zeros((128, 3, 8, 2, 128), np.float32)
    for ct in range(3):
        for g in range(8):
            uy, ux = g // 4, g % 4
            for s in range(2):
                u = (uy + 2 * s) * 4 + ux
                kdiag[:, ct, g, s, :] = ey * w4k[128 * ct:128 * (ct + 1),
                                                u][:, None]
    kdiag8 = kdiag.reshape(128, 3 * 8 * 2 * 128).astype(F8_NP)
    vdiag = np.zeros((128, 3, 9, 128), np.float32)
    for ct in range(3):
        for t9 in range(9):
            w = w3v[128 * ct:128 * (ct + 1), t9][:, None]
            vdiag[:, ct, t9, :] = ey * w
    vdiagb = vdiag.reshape(128, 3 * 9 * 128).astype(BF_NP)

    # wqdw8: pairs [(0,1),(3,4),(6,7),(2,5),(8,-)] as [in, grp, co, s, 128]
    wqdwT = np.transpose(w_q_dw, (1, 2, 3, 0)).reshape(DIM, 9, DIM) * Q_SCALE
    pair_taps = [(0, 1), (3, 4), (6, 7), (2, 5), (8, None)]
    wqdw8 = np.zeros((DIM, 5, 3, 2, 128), np.float32)
    for gi, (ta, tb) in enumerate(pair_taps):
        for co in range(3):
            wqdw8[:, gi, co, 0, :] = wqdwT[:, ta, 128 * co:128 * (co + 1)]
            if tb is not None:
                wqdw8[:, gi, co, 1, :] = wqdwT[:, tb, 128 * co:128 * (co + 1)]
    wqdw8 = wqdw8.reshape(DIM, 5 * 3 * 2 * 128).astype(F8_NP)

    wpT = np.ascontiguousarray(w_proj.T).astype(BF_NP)
    temp_col = np.repeat(temperature, HC)[:, None].astype(np.float32)
    idn = np.eye(128, dtype=BF_NP)
    return dict(wkvv=wkvv, wkv8=wkv8, wq8=wq8, wqdw8=wqdw8, kdiag8=kdiag8,
                vdiagb=vdiagb, wpT=wpT, temp_col=temp_col, idn=idn)


_NC_CACHE = {}


def _get_nc(dbg=False):
    key = bool(dbg)
    if key not in _NC_CACHE:
        _NC_CACHE[key] = build_program()
    return _NC_CACHE[key]


def make_in_maps(x, y, shared):
    x = np.asarray(x, np.float32)
    y = np.asarray(y, np.float32)
    B = x.shape[0]
    in_maps = []
    for b in range(B):
        m = dict(shared)
        xr = np.ascontiguousarray(x[b].reshape(DIM, NPIX))
        m["x_bf"] = xr.astype(BF_NP)
        m["x_f8"] = xr.astype(F8_NP)
        m["y_f8"] = np.ascontiguousarray(y[b].reshape(DIM, NPIX2)).astype(F8_NP)
        in_maps.append(m)
    return in_maps


def kernel(x, y, w_kv, w_kv_dw, w_q, w_q_dw, w_proj, temperature):
    nc = _get_nc(dbg=False)
    shared = _prep_shared(w_kv, w_kv_dw, w_q, w_q_dw, w_proj, temperature)
    in_maps = make_in_maps(x, y, shared)
    res = run_bass_kernel_spmd(nc, in_maps, core_ids=list(range(len(in_maps))))
    out = np.stack([np.asarray(r["out"], dtype=np.float32).reshape(DIM, H, H)
                    for r in res.results])
    return out


# revision 3
# speedup vs baseline: 1.0809x; 1.0809x over previous
"""Trainium2 Bass kernel for nn_Attention_49074296324413 — v4.

Per-core (data-parallel over batch):
  kv = dw3x3(conv1x1(x, w_kv)); k, v = split(kv)
  k  = avgpool2x2(k) [folded 4x4-stride-2 depthwise]
  q  = conv3x3(conv1x1(y, w_q))
  attn = softmax(norm(q) @ norm(k).T * temp); out = w_proj @ (attn @ v)

v4 strategy:
  - fp8 DoubleRow matmuls wherever precision allows: q path (normalized),
    k path (normalized), and the v depthwise as VALUE+RESIDUAL pairs
    (v1 = v8 + r8 exactly compensates fp8 quantization to ~0.1%).
  - all depthwise convs are diagonal-weight DR matmuls on the tensor
    engine, 2 taps (or value+residual) per instruction; diagonals baked
    on host and DMA'd.
  - DMA: one issue per logical tensor via rearranged DRAM APs (partition
    dim from the row-block dim); critical tensors first per ring.
  - PSUM->SBUF copies split Act/DVE; output bf16.
"""
import numpy as np
import ml_dtypes

import concourse.bass as bass
import concourse.tile as tile
from concourse import bacc, mybir
from concourse.bass_utils import run_bass_kernel_spmd

dt = mybir.dt
BF = dt.bfloat16
F32 = dt.float32
FP8 = dt.float8e4
AF = mybir.ActivationFunctionType
OP = mybir.AluOpType
PM = mybir.MatmulPerfMode

DIM = 384
HEADS = 8
HC = DIM // HEADS
CT = DIM // 128
H = 64
NPIX = H * H
PW = H + 2                 # 66
H2 = 32
NPIX2 = H2 * H2

K_SCALE = 8.0              # on wkv8 k half (cancelled by k norm)
Q_SCALE = 50.0             # on wq8 / wqdw8 (cancelled by q norm)
W4K_SCALE = 256.0          # on k diag weights (cancelled by k norm)
W3V_SCALE = 64.0           # on v diag weights (compensated via wpT/64)

# D conv planes: p0=(0,0) p1=(0,1) p2=(0,2) p3=(1,2) shifts of q1pad.
D_GROUPS = [(0, 0), (0, 1), (0, 2), (2, 0)]  # (plane_base, dy) per DR group

BF_NP = ml_dtypes.bfloat16
F8_NP = ml_dtypes.float8_e4m3


def build_program():
    nc = bacc.Bacc("TRN2", target_bir_lowering=False, debug=False)

    xb_d = nc.dram_tensor("x_bf", (DIM, NPIX), BF, kind="ExternalInput")
    x8_d = nc.dram_tensor("x_f8", (DIM, NPIX), FP8, kind="ExternalInput")
    y8_d = nc.dram_tensor("y_f8", (DIM, NPIX2), FP8, kind="ExternalInput")
    wkvv_d = nc.dram_tensor("wkvv", (DIM, DIM), BF, kind="ExternalInput")
    wkv8_d = nc.dram_tensor("wkv8", (DIM, DIM), FP8, kind="ExternalInput")
    wq8_d = nc.dram_tensor("wq8", (DIM, DIM), FP8, kind="ExternalInput")
    wqdw8_d = nc.dram_tensor("wqdw8", (DIM, 5 * 3 * 2 * 128), FP8,
                             kind="ExternalInput")
    kdiag_d = nc.dram_tensor("kdiag8", (128, 3 * 8 * 2 * 128), FP8,
                             kind="ExternalInput")
    vdiag_d = nc.dram_tensor("vdiagb", (128, 3 * 9 * 128), BF,
                             kind="ExternalInput")
    wpT_d = nc.dram_tensor("wpT", (DIM, DIM), BF, kind="ExternalInput")
    temp_d = nc.dram_tensor("temp_col", (DIM, 1), F32, kind="ExternalInput")
    idn_d = nc.dram_tensor("idn", (128, 128), BF, kind="ExternalInput")

    out_d = nc.dram_tensor("out", (DIM, NPIX), BF, kind="ExternalOutput")

    with tile.TileContext(nc) as tc:
        _emit(nc, tc, xb_d, x8_d, y8_d, wkvv_d, wkv8_d, wq8_d, wqdw8_d,
              kdiag_d, vdiag_d, wpT_d, temp_d, idn_d, out_d)
    nc.compile()
    return nc


def _blk(d, p=128):
    """DRAM AP [C*p, N] -> [p, C, N] (partition dim from row blocks)."""
    return d.rearrange("(c p) n -> p c n", p=p)


def _emit(nc, tc, xb_d, x8_d, y8_d, wkvv_d, wkv8_d, wq8_d, wqdw8_d,
          kdiag_d, vdiag_d, wpT_d, temp_d, idn_d, out_d):
    from contextlib import ExitStack
    ctx = ExitStack()

    cst = ctx.enter_context(tc.tile_pool(name="cst", bufs=1))
    big = ctx.enter_context(tc.tile_pool(name="big", bufs=1))
    xvp = ctx.enter_context(tc.tile_pool(name="xvp", bufs=1))
    wrk = ctx.enter_context(tc.tile_pool(name="wrk", bufs=2))
    osb = ctx.enter_context(tc.tile_pool(name="osb", bufs=3))
    ps_big = ctx.enter_context(tc.tile_pool(name="ps_big", bufs=3, space="PSUM"))
    ps_t = ctx.enter_context(tc.tile_pool(name="ps_t", bufs=2, space="PSUM"))

    # ---------------- DMA issues (one per tensor, priority order) ----------
    # sync: y8, wq8, x8, idn, temp, kdiag, vdiag
    y8 = cst.tile([128, CT, NPIX2], FP8, tag="y8", name="y8")
    nc.sync.dma_start(y8[:], _blk(y8_d.ap()))
    wq8p = cst.tile([128, CT, DIM], FP8, tag="wq8", name="wq8")
    nc.sync.dma_start(wq8p[:], _blk(wq8_d.ap()))
    x8 = cst.tile([128, CT, NPIX], FP8, tag="x8", name="x8")
    nc.sync.dma_start(x8[:], _blk(x8_d.ap()))
    idn_t = cst.tile([128, 128], BF, tag="idn", name="idn")
    nc.sync.dma_start(idn_t[:], idn_d.ap())
    tempc = cst.tile([128, CT, 1], F32, tag="tempc", name="tempc")
    nc.sync.dma_start(tempc[:], _blk(temp_d.ap()))
    kdiag = cst.tile([128, CT, 8, 2, 128], FP8, tag="kdiag", name="kdiag")
    nc.sync.dma_start(kdiag[:].rearrange("p a b c d -> p (a b c d)"),
                      kdiag_d.ap())
    vdiag = cst.tile([128, CT, 9, 128], BF, tag="vdiag", name="vdiag")
    nc.sync.dma_start(vdiag[:].rearrange("p a b c -> p (a b c)"),
                      vdiag_d.ap())

    # scalar ring: wkvv first (A-v dep), xb g0/g1, wqdw8, wkv8, wpT
    wkvv = cst.tile([128, CT, DIM], BF, tag="wkvv", name="wkvv")
    nc.scalar.dma_start(wkvv[:], _blk(wkvv_d.ap()))
    xb = xvp.tile([128, CT, NPIX], BF, tag="xv", name="xb")
    for g in range(3):
        nc.scalar.dma_start(xb[:, :, 1024 * g:1024 * (g + 1)],
                            _blk(xb_d.ap()[:, 1024 * g:1024 * (g + 1)]))
    wqdw8 = cst.tile([128, CT, 5, 3, 2, 128], FP8, tag="wqdw8", name="wqdw8")
    nc.scalar.dma_start(wqdw8[:].rearrange("p a b c d e -> p a (b c d e)"),
                        _blk(wqdw8_d.ap()))
    wpT_t = []
    for h in range(HEADS):
        t = cst.tile([HC, DIM], BF, tag=f"wpT{h}", name=f"wpT{h}")
        nc.scalar.dma_start(t[:], wpT_d.ap()[HC * h:HC * (h + 1), :])
        wpT_t.append(t)

    # gpsimd ring: xb g3, wkv8
    for g in range(3, 4):
        nc.gpsimd.dma_start(xb[:, :, 1024 * g:1024 * (g + 1)],
                            _blk(xb_d.ap()[:, 1024 * g:1024 * (g + 1)]))
    wkv8p = cst.tile([128, CT, DIM], FP8, tag="wkv8", name="wkv8")
    nc.gpsimd.dma_start(wkv8p[:], _blk(wkv8_d.ap()))

    # ---------------- padded buffers ----------------
    # vp8: [128, 2(value,residual), 66, 66] fp8
    # kpad8: [128, 2(plane0, rows+2), 66, 66] fp8
    # q1p8: [128, 4 shifted planes, 36, 32] fp8
    vp8, kpad8, q1p8 = [], [], []
    for ct in range(CT):
        t = big.tile([128, PW, PW], BF, tag=f"vp8{ct}")
        nc.gpsimd.memset(t[:, 0, :], 0.0)
        nc.gpsimd.memset(t[:, PW - 1, :], 0.0)
        nc.gpsimd.memset(t[:, 1:PW - 1, 0:1], 0.0)
        nc.gpsimd.memset(t[:, 1:PW - 1, PW - 1:PW], 0.0)
        vp8.append(t)
        t = big.tile([128, 2, PW, PW], FP8, tag=f"kpad8{ct}")
        nc.gpsimd.memset(t[:, 0, 0, :], 0.0)
        nc.gpsimd.memset(t[:, 0, PW - 1, :], 0.0)
        nc.gpsimd.memset(t[:, 0, 1:PW - 1, 0:1], 0.0)
        nc.gpsimd.memset(t[:, 0, 1:PW - 1, PW - 1:PW], 0.0)
        kpad8.append(t)
        t = big.tile([128, 4, 36, 32], FP8, tag=f"q1p8{ct}")
        nc.gpsimd.memset(t[:, 0, 0, :], 0.0)
        nc.gpsimd.memset(t[:, 0, 33:36, :], 0.0)
        nc.gpsimd.memset(t[:, 0, 1:33, 0:1], 0.0)
        nc.gpsimd.memset(t[:, 1, 0, :], 0.0)
        nc.gpsimd.memset(t[:, 1, 33:36, :], 0.0)
        nc.gpsimd.memset(t[:, 2, 0, :], 0.0)
        nc.gpsimd.memset(t[:, 2, 33:36, :], 0.0)
        nc.gpsimd.memset(t[:, 2, 1:33, 31:32], 0.0)
        nc.gpsimd.memset(t[:, 3, 32:36, :], 0.0)
        nc.gpsimd.memset(t[:, 3, 0:32, 31:32], 0.0)
        q1p8.append(t)

    eps_col = cst.tile([128, 1], F32, tag="eps_col", name="eps_col")
    nc.vector.memset(eps_col[:], 1e-24)
    zero_col = cst.tile([128, 1], F32, tag="zero_col", name="zero_col")
    nc.vector.memset(zero_col[:], 0.0)
    # fp8-output warmup (first fp8-dst op on each engine loads ucode ~9us)
    wu = cst.tile([128, 2], FP8, tag="wu", name="wu")
    wub = cst.tile([128, 2], BF, tag="wub", name="wub")
    nc.vector.memset(wub[:], 0.0)
    nc.vector.scalar_tensor_tensor(out=wu[:], in0=wub[:], scalar=1.0,
                                   in1=wub[:], op0=OP.mult, op1=OP.add)
    nc.vector.scalar_tensor_tensor(out=wu[:], in0=wub[:], scalar=1.0,
                                   in1=wub[:], op0=OP.mult, op1=OP.add)
    nc.scalar.activation(wu[:], wub[:], AF.Copy)
    nc.scalar.activation(wu[:], wub[:], AF.Copy)

    # ---------------- Phase C: q1 = W_q @ y -> q1p8 planes (fp8 DR) --------
    for co in range(CT):
        ps = ps_big.tile([128, 1024], F32, tag="ps", name="ps_c")
        for j in range(2):
            rhs2 = y8[:, 0:2, 512 * j:512 * (j + 1)]
            nc.tensor.matmul(ps[:, 512 * j:512 * (j + 1)],
                             wq8p[:, 0:2, 128 * co:128 * (co + 1)],
                             rhs2, start=True, stop=False,
                             perf_mode=PM.DoubleRow)
            nc.tensor.matmul(ps[:, 512 * j:512 * (j + 1)],
                             wq8p[:, 2, 128 * co:128 * (co + 1)],
                             y8[:, 2, 512 * j:512 * (j + 1)],
                             start=False, stop=True)
        pv = ps[:].rearrange("p (a b) -> p a b", a=32)
        nc.scalar.copy(q1p8[co][:, 0, 1:33, 1:32], pv[:, :, 0:31])
        nc.scalar.copy(q1p8[co][:, 1, 1:33, 0:32], pv)
        nc.scalar.copy(q1p8[co][:, 2, 1:33, 0:31], pv[:, :, 1:32])
        # plane3 = plane2 shifted up 1 row (DVE replicate)
        nc.vector.tensor_copy(q1p8[co][:, 3, 0:32, 0:31],
                              q1p8[co][:, 2, 1:33, 0:31])

    # ---------------- Phase A (v half, bf16) + v8/r8 build ----------------
    for g in range(4):
        for co in [3, 4, 5]:
            ct = co % 3
            ps = ps_big.tile([128, 1024], F32, tag="ps", name="ps_av")
            for ci in range(CT):
                for j in range(2):
                    nc.tensor.matmul(
                        ps[:, 512 * j:512 * (j + 1)],
                        wkvv[:, ci, 128 * ct:128 * (ct + 1)],
                        xb[:, ci, 1024 * g + 512 * j:1024 * g + 512 * (j + 1)],
                        start=(ci == 0), stop=(ci == CT - 1))
            r0 = 1 + 16 * g
            pv = ps[:].rearrange("p (a b) -> p a b", a=16)
            # value plane (Act) then residual plane (DVE, reads psum & v8)
            nc.scalar.copy(vp8[ct][:, r0:r0 + 16, 1:65], pv)

    # ---------------- Phase A (k half, fp8 DR, 2-pass LDW reuse) -----------
    for ct in range(CT):
        for gp in range(2):
            pss = [ps_big.tile([128, 1024], F32, tag="ps", name="ps_ak")
                   for _ in range(2)]
            for c4 in range(4):
                off = 2048 * gp + 512 * c4
                nc.tensor.matmul(pss[c4 // 2][:, 512 * (c4 % 2):512 * (c4 % 2 + 1)],
                                 wkv8p[:, 0:2, 128 * ct:128 * (ct + 1)],
                                 x8[:, 0:2, off:off + 512],
                                 start=True, stop=False, perf_mode=PM.DoubleRow)
            for c4 in range(4):
                off = 2048 * gp + 512 * c4
                nc.tensor.matmul(pss[c4 // 2][:, 512 * (c4 % 2):512 * (c4 % 2 + 1)],
                                 wkv8p[:, 2, 128 * ct:128 * (ct + 1)],
                                 x8[:, 2, off:off + 512],
                                 start=False, stop=True)
            for gg in range(2):
                r0 = 1 + 16 * (2 * gp + gg)
                nc.scalar.copy(kpad8[ct][:, 0, r0:r0 + 16, 1:65],
                               pss[gg][:].rearrange("p (a b) -> p a b", a=16))
        nc.vector.tensor_copy(kpad8[ct][:, 1, 0:64, :],
                              kpad8[ct][:, 0, 2:66, :])

    # ---------------- Phase D: q3 3x3 conv, fp8 DR ----------------
    q3 = [big.tile([128, NPIX2], BF, tag=f"q3n{ct}", name=f"q3n{ct}")
          for ct in range(CT)]
    kpT = [big.tile([128, DIM], BF, tag=f"kpT{pt}", name=f"kpT{pt}")
           for pt in range(8)]
    q3T = [big.tile([128, DIM], BF, tag=f"q3T{pt}", name=f"q3T{pt}")
           for pt in range(8)]
    kp16 = [big.tile([128, NPIX2], BF, tag=f"kp16{ct}", name=f"kp16{ct}")
            for ct in range(CT)]

    for co in range(CT):
        ps = ps_big.tile([128, 1024], F32, tag="ps", name="ps_q3")
        for ci in range(CT):
            first = (ci == 0)
            for gi, (pl0, dy) in enumerate(D_GROUPS):
                for j in range(2):
                    rhs = q1p8[ci][:, pl0:pl0 + 2,
                                   16 * j + dy:16 * j + dy + 16, :]
                    nc.tensor.matmul(
                        ps[:, 512 * j:512 * (j + 1)],
                        wqdw8[:, ci, gi, co, :, :], rhs,
                        start=(first and gi == 0), stop=False,
                        perf_mode=PM.DoubleRow)
            for j in range(2):
                nc.tensor.matmul(
                    ps[:, 512 * j:512 * (j + 1)],
                    wqdw8[:, ci, 4, co, 0, :],
                    q1p8[ci][:, 2, 16 * j + 2:16 * j + 18, :],
                    start=False, stop=(ci == CT - 1))
        nc.scalar.copy(q3[co][:], ps[:])
        nrm2 = wrk.tile([128, 1], F32, tag="nrm2q", name="nrm2q")
        sqq = wrk.tile([128, NPIX2], BF, tag="sqq", name="sqq")
        nc.scalar.activation(sqq[:], q3[co][:], AF.Square, bias=zero_col[:],
                             accum_out=nrm2[:])
        nrm = wrk.tile([128, 1], F32, tag="nrmq", name="nrmq")
        nc.scalar.activation(nrm[:], nrm2[:], AF.Sqrt, bias=eps_col[:])
        inv = wrk.tile([128, 1], F32, tag="invq", name="invq")
        nc.vector.reciprocal(inv[:], nrm[:])
        invt = wrk.tile([128, 1], F32, tag="invqt", name="invqt")
        nc.vector.tensor_mul(invt[:], inv[:], tempc[:, co, :])
        nc.vector.tensor_scalar_mul(q3[co][:], q3[co][:], invt[:])

    # ---------------- k depthwise+pool: fp8 DR diag ----------------
    for ct in range(CT):
        psk = ps_big.tile([128, 1024], F32, tag="ps", name="ps_k")
        kv = kpad8[ct][:].rearrange(
            "p pl (rp two) (cp ctwo) -> p pl rp two cp ctwo", two=2, ctwo=2)
        for g in range(8):
            uy, ux = g // 4, g % 4
            for h2 in range(2):
                ph = psk[:, 512 * h2:512 * (h2 + 1)].rearrange(
                    "p (a b) -> p a b", a=16)
                rhs = kv[:, 0:2, 16 * h2:16 * h2 + 16, uy,
                         ux // 2:ux // 2 + 32, ux % 2]
                nc.tensor.matmul(ph, kdiag[:, ct, g, :, :], rhs,
                                 start=(g == 0), stop=(g == 7),
                                 perf_mode=PM.DoubleRow)
        nrm2 = wrk.tile([128, 1], F32, tag="nrm2k", name="nrm2k")
        sqk = wrk.tile([128, NPIX2], BF, tag="sqk", name="sqk")
        nc.scalar.activation(sqk[:], psk[:], AF.Square, bias=zero_col[:],
                             accum_out=nrm2[:])
        nrm = wrk.tile([128, 1], F32, tag="nrmk", name="nrmk")
        nc.scalar.activation(nrm[:], nrm2[:], AF.Sqrt, bias=eps_col[:])
        inv = wrk.tile([128, 1], F32, tag="invk", name="invk")
        nc.vector.reciprocal(inv[:], nrm[:])
        nc.vector.tensor_scalar_mul(kp16[ct][:], psk[:], inv[:])

    # ---------------- v depthwise: value+residual fp8 DR diag --------------
    # reuses xb's SBUF (xv tag, bufs=1): all xb readers finish in phase A-v
    vdw_all = xvp.tile([128, CT, NPIX], BF, tag="xv", name="vdw")
    v_dw = [vdw_all[:, ct, :] for ct in range(CT)]

    def _vdw_ct(ct):
        for qp in range(2):
            pss = [ps_big.tile([128, 1024], F32, tag="ps", name="ps_v")
                   for _ in range(2)]
            for t9 in range(9):
                dy, dx = t9 // 3, t9 % 3
                for c4 in range(4):
                    pj = pss[c4 // 2][:, 512 * (c4 % 2):512 * (c4 % 2 + 1)]
                    r0 = 32 * qp + 8 * c4 + dy
                    rhs = vp8[ct][:, r0:r0 + 8, dx:dx + H]
                    nc.tensor.matmul(pj.rearrange("p (a b) -> p a b", a=8),
                                     vdiag[:, ct, t9, :], rhs,
                                     start=(t9 == 0), stop=(t9 == 8))
            for gg in range(2):
                q = 2 * qp + gg
                if gg == 0:
                    nc.scalar.copy(v_dw[ct][:, 1024 * q:1024 * (q + 1)],
                                   pss[gg][:])
                else:
                    nc.vector.tensor_copy(v_dw[ct][:, 1024 * q:1024 * (q + 1)],
                                          pss[gg][:])

    # interleave: vdw ct0, transposes, vdw ct1, (F emitted after), vdw ct2
    _vdw_ct(0)
    for ct in range(CT):
        for pt in range(8):
            pst = ps_t.tile([128, 128], BF, tag="ps_t", name="ps_tr")
            nc.tensor.transpose(pst[:], kp16[ct][:, 128 * pt:128 * (pt + 1)],
                                idn_t[:])
            nc.vector.tensor_copy(kpT[pt][:, 128 * ct:128 * (ct + 1)], pst[:])
    for ct in range(CT):
        for pt in range(8):
            pst = ps_t.tile([128, 128], BF, tag="ps_t", name="ps_trq")
            nc.tensor.transpose(pst[:], q3[ct][:, 128 * pt:128 * (pt + 1)],
                                idn_t[:])
            nc.vector.tensor_copy(q3T[pt][:, 128 * ct:128 * (ct + 1)], pst[:])
    _vdw_ct(1)
    _vdw_ct(2)

    # ---------------- F: QK + softmax + M (per head) ----------------
    mst = [big.tile([128, DIM], BF, tag=f"mst{ct}", name=f"mst{ct}")
           for ct in range(CT)]
    att_n = []
    for h in range(HEADS):
        cs = slice(HC * h, HC * (h + 1))
        pa = ps_t.tile([HC, HC], F32, tag="ps_t", name="ps_at")
        for pt in range(8):
            nc.tensor.matmul(pa[:], q3T[pt][:, cs], kpT[pt][:, cs],
                             start=(pt == 0), stop=(pt == 7))
        ae = wrk.tile([HC, HC], BF, tag=f"ae{h % 2}", name=f"ae{h % 2}", bufs=2)
        nc.scalar.activation(ae[:], pa[:], AF.Exp, bias=zero_col[0:HC, :])
        zs = wrk.tile([HC, 1], F32, tag="zs", name="zs")
        nc.vector.tensor_reduce(zs[:], ae[:], axis=mybir.AxisListType.X,
                                op=OP.add)
        zi = wrk.tile([HC, 1], F32, tag="zi", name="zi")
        nc.vector.reciprocal(zi[:], zs[:])
        an = wrk.tile([HC, HC], BF, tag=f"an{h}", name=f"an{h}")
        nc.vector.tensor_scalar_mul(an[:], ae[:], zi[:])
        att_n.append(an)
    for h in range(HEADS):
        pm = ps_t.tile([HC, DIM], F32, tag="ps_t", name="ps_M")
        nc.tensor.matmul(pm[:], att_n[h][:], wpT_t[h][:], start=True, stop=True)
        stg = wrk.tile([HC, DIM], BF, tag=f"stg{h % 2}", name=f"stg{h % 2}")
        nc.vector.tensor_copy(stg[:], pm[:])
        g0 = HC * h
        t0, o0 = divmod(g0, 128)
        n0 = min(128 - o0, HC)
        nc.sync.dma_start(mst[t0][o0:o0 + n0, :], stg[0:n0, :])
        if n0 < HC:
            nc.sync.dma_start(mst[t0 + 1][0:HC - n0, :], stg[n0:HC, :])

    # ---------------- H: out = Mst.T @ v_dw ----------------
    for ob in range(CT):
        for g4 in range(4):
            ps = ps_big.tile([128, 1024], F32, tag="ps", name="ps_h")
            for ctd in range(CT):
                for j in range(2):
                    nc.tensor.matmul(
                        ps[:, 512 * j:512 * (j + 1)],
                        mst[ctd][:, 128 * ob:128 * (ob + 1)],
                        v_dw[ctd][:, 1024 * g4 + 512 * j:1024 * g4 + 512 * (j + 1)],
                        start=(ctd == 0), stop=(ctd == CT - 1))
            ot = osb.tile([128, 1024], BF, tag="osb", name="osb", bufs=3)
            if g4 % 2 == 0:
                nc.scalar.copy(ot[:], ps[:])
            else:
                nc.vector.tensor_copy(ot[:], ps[:])
            eng = nc.sync if g4 % 2 == 0 else nc.scalar
            eng.dma_start(out_d.ap()[128 * ob:128 * (ob + 1),
                                     1024 * g4:1024 * (g4 + 1)], ot[:])
    ctx.close()


# ======================= host-side wrapper =======================

def _prep_shared(w_kv, w_kv_dw, w_q, w_q_dw, w_proj, temperature):
    w_kv = np.asarray(w_kv, np.float32)[:, :, 0, 0]          # [768, 384]
    w_kv_dw = np.asarray(w_kv_dw, np.float32)[:, 0]          # [768, 3, 3]
    w_q = np.asarray(w_q, np.float32)[:, :, 0, 0]            # [384, 384]
    w_q_dw = np.asarray(w_q_dw, np.float32)                  # [384, 384, 3, 3]
    w_proj = np.asarray(w_proj, np.float32)[:, :, 0, 0]      # [384, 384]
    temperature = np.asarray(temperature, np.float32).reshape(HEADS)

    # wkvv: v-half 1x1 weights, [in 384, out 384] transposed, bf16
    wkvv = np.ascontiguousarray(w_kv[DIM:].T).astype(BF_NP)
    # wkv8: k-half fp8 [in 384, ci-major 3 x out 384]; scale K_SCALE
    wk = w_kv[:DIM].T * K_SCALE                              # [in, out]
    wkv8 = np.ascontiguousarray(wk).astype(F8_NP)            # rows = in
    # wq8 similarly, scaled Q_SCALE
    wq8 = np.ascontiguousarray(w_q.T * Q_SCALE).astype(F8_NP)

    w3v = w_kv_dw[DIM:].reshape(DIM, 9)
    w3k = w_kv_dw[:DIM]
    w4k = np.zeros((DIM, 4, 4), np.float32)
    for uy in range(4):
        for ux in range(4):
            acc = np.zeros(DIM, np.float32)
            for dy in range(2):
                for dx in range(2):
                    ky, kx = uy - dy, ux - dx
                    if 0 <= ky < 3 and 0 <= kx < 3:
                        acc += w3k[:, ky, kx]
            w4k[:, uy, ux] = 0.25 * acc * W4K_SCALE
    w4k = w4k.reshape(DIM, 16)

    # diag weights baked on host
    ey = np.eye(128, dtype=np.float32)
    kdiag = np.zeros((128, 3, 8, 2, 128), np.float32)
    for ct in range(3):
        for g in range(8):
            uy, ux = g // 4, g % 4
            for s in range(2):
                u = (uy + 2 * s) * 4 + ux
                kdiag[:, ct, g, s, :] = ey * w4k[128 * ct:128 * (ct + 1),
                                                u][:, None]
    kdiag8 = kdiag.reshape(128, 3 * 8 * 2 * 128).astype(F8_NP)
    vdiag = np.zeros((128, 3, 9, 128), np.float32)
    for ct in range(3):
        for t9 in range(9):
            w = w3v[128 * ct:128 * (ct + 1), t9][:, None]
            vdiag[:, ct, t9, :] = ey * w
    vdiagb = vdiag.reshape(128, 3 * 9 * 128).astype(BF_NP)

    # wqdw8: pairs [(0,1),(3,4),(6,7),(2,5),(8,-)] as [in, grp, co, s, 128]
    wqdwT = np.transpose(w_q_dw, (1, 2, 3, 0)).reshape(DIM, 9, DIM) * Q_SCALE
    pair_taps = [(0, 1), (3, 4), (6, 7), (2, 5), (8, None)]
    wqdw8 = np.zeros((DIM, 5, 3, 2, 128), np.float32)
    for gi, (ta, tb) in enumerate(pair_taps):
        for co in range(3):
            wqdw8[:, gi, co, 0, :] = wqdwT[:, ta, 128 * co:128 * (co + 1)]
            if tb is not None:
                wqdw8[:, gi, co, 1, :] = wqdwT[:, tb, 128 * co:128 * (co + 1)]
    wqdw8 = wqdw8.reshape(DIM, 5 * 3 * 2 * 128).astype(F8_NP)

    wpT = np.ascontiguousarray(w_proj.T).astype(BF_NP)
    temp_col = np.repeat(temperature, HC)[:, None].astype(np.float32)
    idn = np.eye(128, dtype=BF_NP)
    return dict(wkvv=wkvv, wkv8=wkv8, wq8=wq8, wqdw8=wqdw8, kdiag8=kdiag8,
                vdiagb=vdiagb, wpT=wpT, temp_col=temp_col, idn=idn)


_NC_CACHE = {}


def _get_nc(dbg=False):
    key = bool(dbg)
    if key not in _NC_CACHE:
        _NC_CACHE[key] = build_program()
    return _NC_CACHE[key]


def make_in_maps(x, y, shared):
    x = np.asarray(x, np.float32)
    y = np.asarray(y, np.float32)
    B = x.shape[0]
    in_maps = []
    for b in range(B):
        m = dict(shared)
        xr = np.ascontiguousarray(x[b].reshape(DIM, NPIX))
        m["x_bf"] = xr.astype(BF_NP)
        m["x_f8"] = xr.astype(F8_NP)
        m["y_f8"] = np.ascontiguousarray(y[b].reshape(DIM, NPIX2)).astype(F8_NP)
        in_maps.append(m)
    return in_maps


def kernel(x, y, w_kv, w_kv_dw, w_q, w_q_dw, w_proj, temperature):
    nc = _get_nc(dbg=False)
    shared = _prep_shared(w_kv, w_kv_dw, w_q, w_q_dw, w_proj, temperature)
    in_maps = make_in_maps(x, y, shared)
    res = run_bass_kernel_spmd(nc, in_maps, core_ids=list(range(len(in_maps))))
    out = np.stack([np.asarray(r["out"], dtype=np.float32).reshape(DIM, H, H)
                    for r in res.results])
    return out


# revision 4
# speedup vs baseline: 1.1073x; 1.0244x over previous
"""Trainium2 Bass kernel for nn_Attention_49074296324413 — v4.

Per-core (data-parallel over batch):
  kv = dw3x3(conv1x1(x, w_kv)); k, v = split(kv)
  k  = avgpool2x2(k) [folded 4x4-stride-2 depthwise]
  q  = conv3x3(conv1x1(y, w_q))
  attn = softmax(norm(q) @ norm(k).T * temp); out = w_proj @ (attn @ v)

v4 strategy:
  - fp8 DoubleRow matmuls wherever precision allows: q path (normalized),
    k path (normalized), and the v depthwise as VALUE+RESIDUAL pairs
    (v1 = v8 + r8 exactly compensates fp8 quantization to ~0.1%).
  - all depthwise convs are diagonal-weight DR matmuls on the tensor
    engine, 2 taps (or value+residual) per instruction; diagonals baked
    on host and DMA'd.
  - DMA: one issue per logical tensor via rearranged DRAM APs (partition
    dim from the row-block dim); critical tensors first per ring.
  - PSUM->SBUF copies split Act/DVE; output bf16.
"""
import numpy as np
import ml_dtypes

import concourse.bass as bass
import concourse.tile as tile
from concourse import bacc, mybir
from concourse.bass_utils import run_bass_kernel_spmd

dt = mybir.dt
BF = dt.bfloat16
F32 = dt.float32
FP8 = dt.float8e4
AF = mybir.ActivationFunctionType
OP = mybir.AluOpType
PM = mybir.MatmulPerfMode

DIM = 384
HEADS = 8
HC = DIM // HEADS
CT = DIM // 128
H = 64
NPIX = H * H
PW = H + 2                 # 66
H2 = 32
NPIX2 = H2 * H2

K_SCALE = 8.0              # on wkv8 k half (cancelled by k norm)
Q_SCALE = 50.0             # on wq8 / wqdw8 (cancelled by q norm)
W4K_SCALE = 256.0          # on k diag weights (cancelled by k norm)
W3V_SCALE = 64.0           # on v diag weights (compensated via wpT/64)

# D conv planes: p0=(0,0) p1=(0,1) p2=(0,2) p3=(1,2) shifts of q1pad.
D_GROUPS = [(0, 0), (0, 1), (0, 2), (2, 0)]  # (plane_base, dy) per DR group

BF_NP = ml_dtypes.bfloat16
F8_NP = ml_dtypes.float8_e4m3


def build_program():
    nc = bacc.Bacc("TRN2", target_bir_lowering=False, debug=False)

    xb_d = nc.dram_tensor("x_bf", (DIM, NPIX), BF, kind="ExternalInput")
    x8_d = nc.dram_tensor("x_f8", (DIM, NPIX), FP8, kind="ExternalInput")
    y8_d = nc.dram_tensor("y_f8", (DIM, NPIX2), FP8, kind="ExternalInput")
    wkvv_d = nc.dram_tensor("wkvv", (DIM, DIM), BF, kind="ExternalInput")
    wkv8_d = nc.dram_tensor("wkv8", (DIM, DIM), FP8, kind="ExternalInput")
    wq8_d = nc.dram_tensor("wq8", (DIM, DIM), FP8, kind="ExternalInput")
    wqdw8_d = nc.dram_tensor("wqdw8", (DIM, 5 * 3 * 2 * 128), FP8,
                             kind="ExternalInput")
    kdiag_d = nc.dram_tensor("kdiag8", (128, 3 * 8 * 2 * 128), FP8,
                             kind="ExternalInput")
    vdiag_d = nc.dram_tensor("vdiagb", (128, 3 * 9 * 128), BF,
                             kind="ExternalInput")
    wpT_d = nc.dram_tensor("wpT", (DIM, DIM), BF, kind="ExternalInput")
    temp_d = nc.dram_tensor("temp_col", (DIM, 1), F32, kind="ExternalInput")
    idn_d = nc.dram_tensor("idn", (128, 128), BF, kind="ExternalInput")

    out_d = nc.dram_tensor("out", (DIM, NPIX), BF, kind="ExternalOutput")

    with tile.TileContext(nc) as tc:
        _emit(nc, tc, xb_d, x8_d, y8_d, wkvv_d, wkv8_d, wq8_d, wqdw8_d,
              kdiag_d, vdiag_d, wpT_d, temp_d, idn_d, out_d)
    nc.compile()
    return nc


def _blk(d, p=128):
    """DRAM AP [C*p, N] -> [p, C, N] (partition dim from row blocks)."""
    return d.rearrange("(c p) n -> p c n", p=p)


def _emit(nc, tc, xb_d, x8_d, y8_d, wkvv_d, wkv8_d, wq8_d, wqdw8_d,
          kdiag_d, vdiag_d, wpT_d, temp_d, idn_d, out_d):
    from contextlib import ExitStack
    ctx = ExitStack()

    cst = ctx.enter_context(tc.tile_pool(name="cst", bufs=1))
    big = ctx.enter_context(tc.tile_pool(name="big", bufs=1))
    xvp = ctx.enter_context(tc.tile_pool(name="xvp", bufs=1))
    wrk = ctx.enter_context(tc.tile_pool(name="wrk", bufs=2))
    osb = ctx.enter_context(tc.tile_pool(name="osb", bufs=3))
    ps_big = ctx.enter_context(tc.tile_pool(name="ps_big", bufs=3, space="PSUM"))
    ps_t = ctx.enter_context(tc.tile_pool(name="ps_t", bufs=2, space="PSUM"))

    # ---------------- DMA issues (one per tensor, priority order) ----------
    # sync: y8, wq8, x8, idn, temp, kdiag, vdiag
    y8 = cst.tile([128, CT, NPIX2], FP8, tag="y8", name="y8")
    nc.sync.dma_start(y8[:], _blk(y8_d.ap()))
    wq8p = cst.tile([128, CT, DIM], FP8, tag="wq8", name="wq8")
    nc.sync.dma_start(wq8p[:], _blk(wq8_d.ap()))
    x8 = cst.tile([128, CT, NPIX], FP8, tag="x8", name="x8")
    nc.sync.dma_start(x8[:], _blk(x8_d.ap()))
    idn_t = cst.tile([128, 128], BF, tag="idn", name="idn")
    nc.sync.dma_start(idn_t[:], idn_d.ap())
    tempc = cst.tile([128, CT, 1], F32, tag="tempc", name="tempc")
    nc.sync.dma_start(tempc[:], _blk(temp_d.ap()))
    kdiag = cst.tile([128, CT, 8, 2, 128], FP8, tag="kdiag", name="kdiag")
    nc.sync.dma_start(kdiag[:].rearrange("p a b c d -> p (a b c d)"),
                      kdiag_d.ap())
    vdiag = cst.tile([128, CT, 9, 128], BF, tag="vdiag", name="vdiag")
    nc.sync.dma_start(vdiag[:].rearrange("p a b c -> p (a b c)"),
                      vdiag_d.ap())

    # scalar ring: wkvv first (A-v dep), xb g0/g1, wqdw8, wkv8, wpT
    wkvv = cst.tile([128, CT, DIM], BF, tag="wkvv", name="wkvv")
    nc.scalar.dma_start(wkvv[:], _blk(wkvv_d.ap()))
    xb = xvp.tile([128, CT, NPIX], BF, tag="xv", name="xb")
    for g in range(3):
        nc.scalar.dma_start(xb[:, :, 1024 * g:1024 * (g + 1)],
                            _blk(xb_d.ap()[:, 1024 * g:1024 * (g + 1)]))

    # gpsimd ring: xb g3, wkv8
    for g in range(3, 4):
        nc.gpsimd.dma_start(xb[:, :, 1024 * g:1024 * (g + 1)],
                            _blk(xb_d.ap()[:, 1024 * g:1024 * (g + 1)]))
    wkv8p = cst.tile([128, CT, DIM], FP8, tag="wkv8", name="wkv8")
    nc.gpsimd.dma_start(wkv8p[:], _blk(wkv8_d.ap()))

    # ---------------- padded buffers ----------------
    # vp8: [128, 2(value,residual), 66, 66] fp8
    # kpad8: [128, 2(plane0, rows+2), 66, 66] fp8
    # q1p8: [128, 4 shifted planes, 36, 32] fp8
    vp8, kpad8, q1p8 = [], [], []
    for ct in range(CT):
        t = big.tile([128, PW, PW], BF, tag=f"vp8{ct}")
        nc.gpsimd.memset(t[:, 0, :], 0.0)
        nc.gpsimd.memset(t[:, PW - 1, :], 0.0)
        nc.gpsimd.memset(t[:, 1:PW - 1, 0:1], 0.0)
        nc.gpsimd.memset(t[:, 1:PW - 1, PW - 1:PW], 0.0)
        vp8.append(t)
        t = big.tile([128, 2, PW, PW], FP8, tag=f"kpad8{ct}")
        nc.gpsimd.memset(t[:, 0, 0, :], 0.0)
        nc.gpsimd.memset(t[:, 0, PW - 1, :], 0.0)
        nc.gpsimd.memset(t[:, 0, 1:PW - 1, 0:1], 0.0)
        nc.gpsimd.memset(t[:, 0, 1:PW - 1, PW - 1:PW], 0.0)
        kpad8.append(t)
        t = big.tile([128, 4, 36, 32], FP8, tag=f"q1p8{ct}")
        nc.gpsimd.memset(t[:, 0, 0, :], 0.0)
        nc.gpsimd.memset(t[:, 0, 33:36, :], 0.0)
        nc.gpsimd.memset(t[:, 0, 1:33, 0:1], 0.0)
        nc.gpsimd.memset(t[:, 1, 0, :], 0.0)
        nc.gpsimd.memset(t[:, 1, 33:36, :], 0.0)
        nc.gpsimd.memset(t[:, 2, 0, :], 0.0)
        nc.gpsimd.memset(t[:, 2, 33:36, :], 0.0)
        nc.gpsimd.memset(t[:, 2, 1:33, 31:32], 0.0)
        nc.gpsimd.memset(t[:, 3, 32:36, :], 0.0)
        nc.gpsimd.memset(t[:, 3, 0:32, 31:32], 0.0)
        q1p8.append(t)

    eps_col = cst.tile([128, 1], F32, tag="eps_col", name="eps_col")
    nc.vector.memset(eps_col[:], 1e-24)
    zero_col = cst.tile([128, 1], F32, tag="zero_col", name="zero_col")
    nc.vector.memset(zero_col[:], 0.0)
    # fp8-output warmup (first fp8-dst op on each engine loads ucode ~9us)
    wu = cst.tile([128, 2], FP8, tag="wu", name="wu")
    wub = cst.tile([128, 2], BF, tag="wub", name="wub")
    nc.vector.memset(wub[:], 0.0)
    nc.vector.scalar_tensor_tensor(out=wu[:], in0=wub[:], scalar=1.0,
                                   in1=wub[:], op0=OP.mult, op1=OP.add)
    nc.vector.scalar_tensor_tensor(out=wu[:], in0=wub[:], scalar=1.0,
                                   in1=wub[:], op0=OP.mult, op1=OP.add)
    nc.scalar.activation(wu[:], wub[:], AF.Copy)
    nc.scalar.activation(wu[:], wub[:], AF.Copy)

    # ---------------- Phase C: q1 = W_q @ y -> q1p8 planes (fp8 DR) --------
    for co in range(CT):
        ps = ps_big.tile([128, 1024], F32, tag="ps", name="ps_c")
        for j in range(2):
            rhs2 = y8[:, 0:2, 512 * j:512 * (j + 1)]
            nc.tensor.matmul(ps[:, 512 * j:512 * (j + 1)],
                             wq8p[:, 0:2, 128 * co:128 * (co + 1)],
                             rhs2, start=True, stop=False,
                             perf_mode=PM.DoubleRow)
            nc.tensor.matmul(ps[:, 512 * j:512 * (j + 1)],
                             wq8p[:, 2, 128 * co:128 * (co + 1)],
                             y8[:, 2, 512 * j:512 * (j + 1)],
                             start=False, stop=True)
        pv = ps[:].rearrange("p (a b) -> p a b", a=32)
        nc.scalar.copy(q1p8[co][:, 0, 1:33, 1:32], pv[:, :, 0:31])
        nc.scalar.copy(q1p8[co][:, 1, 1:33, 0:32], pv)
        nc.scalar.copy(q1p8[co][:, 2, 1:33, 0:31], pv[:, :, 1:32])
        # plane3 = plane2 shifted up 1 row (DVE replicate)
        nc.vector.tensor_copy(q1p8[co][:, 3, 0:32, 0:31],
                              q1p8[co][:, 2, 1:33, 0:31])

    # deferred DMA issues: ring has drained by now; needed much later
    wqdw8 = cst.tile([128, CT, 5, 3, 2, 128], FP8, tag="wqdw8", name="wqdw8")
    nc.scalar.dma_start(wqdw8[:].rearrange("p a b c d e -> p a (b c d e)"),
                        _blk(wqdw8_d.ap()))

    # ---------------- Phase A (v half, bf16) + v8/r8 build ----------------
    for g in range(4):
        for co in [3, 4, 5]:
            ct = co % 3
            ps = ps_big.tile([128, 1024], F32, tag="ps", name="ps_av")
            for ci in range(CT):
                for j in range(2):
                    nc.tensor.matmul(
                        ps[:, 512 * j:512 * (j + 1)],
                        wkvv[:, ci, 128 * ct:128 * (ct + 1)],
                        xb[:, ci, 1024 * g + 512 * j:1024 * g + 512 * (j + 1)],
                        start=(ci == 0), stop=(ci == CT - 1))
            r0 = 1 + 16 * g
            pv = ps[:].rearrange("p (a b) -> p a b", a=16)
            # value plane (Act) then residual plane (DVE, reads psum & v8)
            nc.scalar.copy(vp8[ct][:, r0:r0 + 16, 1:65], pv)

    # deferred: wpT needed only at phase F
    wpT_t = []
    for h in range(HEADS):
        t = cst.tile([HC, DIM], BF, tag=f"wpT{h}", name=f"wpT{h}")
        nc.scalar.dma_start(t[:], wpT_d.ap()[HC * h:HC * (h + 1), :])
        wpT_t.append(t)

    # ---------------- Phase A (k half, fp8 DR, 2-pass LDW reuse) -----------
    for ct in range(CT):
        for gp in range(2):
            pss = [ps_big.tile([128, 1024], F32, tag="ps", name="ps_ak")
                   for _ in range(2)]
            for c4 in range(4):
                off = 2048 * gp + 512 * c4
                nc.tensor.matmul(pss[c4 // 2][:, 512 * (c4 % 2):512 * (c4 % 2 + 1)],
                                 wkv8p[:, 0:2, 128 * ct:128 * (ct + 1)],
                                 x8[:, 0:2, off:off + 512],
                                 start=True, stop=False, perf_mode=PM.DoubleRow)
            for c4 in range(4):
                off = 2048 * gp + 512 * c4
                nc.tensor.matmul(pss[c4 // 2][:, 512 * (c4 % 2):512 * (c4 % 2 + 1)],
                                 wkv8p[:, 2, 128 * ct:128 * (ct + 1)],
                                 x8[:, 2, off:off + 512],
                                 start=False, stop=True)
            for gg in range(2):
                r0 = 1 + 16 * (2 * gp + gg)
                nc.scalar.copy(kpad8[ct][:, 0, r0:r0 + 16, 1:65],
                               pss[gg][:].rearrange("p (a b) -> p a b", a=16))
        nc.vector.tensor_copy(kpad8[ct][:, 1, 0:64, :],
                              kpad8[ct][:, 0, 2:66, :])

    # ---------------- Phase D: q3 3x3 conv, fp8 DR ----------------
    q3 = [big.tile([128, NPIX2], BF, tag=f"q3n{ct}", name=f"q3n{ct}")
          for ct in range(CT)]
    kpT = [big.tile([128, DIM], BF, tag=f"kpT{pt}", name=f"kpT{pt}")
           for pt in range(8)]
    q3T = [big.tile([128, DIM], BF, tag=f"q3T{pt}", name=f"q3T{pt}")
           for pt in range(8)]
    kp16 = [big.tile([128, NPIX2], BF, tag=f"kp16{ct}", name=f"kp16{ct}")
            for ct in range(CT)]

    for co in range(CT):
        ps = ps_big.tile([128, 1024], F32, tag="ps", name="ps_q3")
        for ci in range(CT):
            first = (ci == 0)
            for gi, (pl0, dy) in enumerate(D_GROUPS):
                for j in range(2):
                    rhs = q1p8[ci][:, pl0:pl0 + 2,
                                   16 * j + dy:16 * j + dy + 16, :]
                    nc.tensor.matmul(
                        ps[:, 512 * j:512 * (j + 1)],
                        wqdw8[:, ci, gi, co, :, :], rhs,
                        start=(first and gi == 0), stop=False,
                        perf_mode=PM.DoubleRow)
            for j in range(2):
                nc.tensor.matmul(
                    ps[:, 512 * j:512 * (j + 1)],
                    wqdw8[:, ci, 4, co, 0, :],
                    q1p8[ci][:, 2, 16 * j + 2:16 * j + 18, :],
                    start=False, stop=(ci == CT - 1))
        nc.scalar.copy(q3[co][:], ps[:])
        nrm2 = wrk.tile([128, 1], F32, tag="nrm2q", name="nrm2q")
        sqq = wrk.tile([128, NPIX2], BF, tag="sqq", name="sqq")
        nc.scalar.activation(sqq[:], q3[co][:], AF.Square, bias=zero_col[:],
                             accum_out=nrm2[:])
        nrm = wrk.tile([128, 1], F32, tag="nrmq", name="nrmq")
        nc.scalar.activation(nrm[:], nrm2[:], AF.Sqrt, bias=eps_col[:])
        inv = wrk.tile([128, 1], F32, tag="invq", name="invq")
        nc.vector.reciprocal(inv[:], nrm[:])
        invt = wrk.tile([128, 1], F32, tag="invqt", name="invqt")
        nc.vector.tensor_mul(invt[:], inv[:], tempc[:, co, :])
        nc.vector.tensor_scalar_mul(q3[co][:], q3[co][:], invt[:])

    # ---------------- k depthwise+pool: fp8 DR diag ----------------
    for ct in range(CT):
        psk = ps_big.tile([128, 1024], F32, tag="ps", name="ps_k")
        kv = kpad8[ct][:].rearrange(
            "p pl (rp two) (cp ctwo) -> p pl rp two cp ctwo", two=2, ctwo=2)
        for g in range(8):
            uy, ux = g // 4, g % 4
            for h2 in range(2):
                ph = psk[:, 512 * h2:512 * (h2 + 1)].rearrange(
                    "p (a b) -> p a b", a=16)
                rhs = kv[:, 0:2, 16 * h2:16 * h2 + 16, uy,
                         ux // 2:ux // 2 + 32, ux % 2]
                nc.tensor.matmul(ph, kdiag[:, ct, g, :, :], rhs,
                                 start=(g == 0), stop=(g == 7),
                                 perf_mode=PM.DoubleRow)
        nrm2 = wrk.tile([128, 1], F32, tag="nrm2k", name="nrm2k")
        sqk = wrk.tile([128, NPIX2], BF, tag="sqk", name="sqk")
        nc.scalar.activation(sqk[:], psk[:], AF.Square, bias=zero_col[:],
                             accum_out=nrm2[:])
        nrm = wrk.tile([128, 1], F32, tag="nrmk", name="nrmk")
        nc.scalar.activation(nrm[:], nrm2[:], AF.Sqrt, bias=eps_col[:])
        inv = wrk.tile([128, 1], F32, tag="invk", name="invk")
        nc.vector.reciprocal(inv[:], nrm[:])
        nc.vector.tensor_scalar_mul(kp16[ct][:], psk[:], inv[:])

    # ---------------- v depthwise: value+residual fp8 DR diag --------------
    # reuses xb's SBUF (xv tag, bufs=1): all xb readers finish in phase A-v
    vdw_all = xvp.tile([128, CT, NPIX], BF, tag="xv", name="vdw")
    v_dw = [vdw_all[:, ct, :] for ct in range(CT)]

    def _vdw_ct(ct):
        for qp in range(2):
            pss = [ps_big.tile([128, 1024], F32, tag="ps", name="ps_v")
                   for _ in range(2)]
            for t9 in range(9):
                dy, dx = t9 // 3, t9 % 3
                for c4 in range(4):
                    pj = pss[c4 // 2][:, 512 * (c4 % 2):512 * (c4 % 2 + 1)]
                    r0 = 32 * qp + 8 * c4 + dy
                    rhs = vp8[ct][:, r0:r0 + 8, dx:dx + H]
                    nc.tensor.matmul(pj.rearrange("p (a b) -> p a b", a=8),
                                     vdiag[:, ct, t9, :], rhs,
                                     start=(t9 == 0), stop=(t9 == 8))
            for gg in range(2):
                q = 2 * qp + gg
                if gg == 0:
                    nc.scalar.copy(v_dw[ct][:, 1024 * q:1024 * (q + 1)],
                                   pss[gg][:])
                else:
                    nc.vector.tensor_copy(v_dw[ct][:, 1024 * q:1024 * (q + 1)],
                                          pss[gg][:])

    # interleave: vdw ct0, transposes, vdw ct1, (F emitted after), vdw ct2
    _vdw_ct(0)
    for ct in range(CT):
        for pt in range(8):
            pst = ps_t.tile([128, 128], BF, tag="ps_t", name="ps_tr")
            nc.tensor.transpose(pst[:], kp16[ct][:, 128 * pt:128 * (pt + 1)],
                                idn_t[:])
            nc.vector.tensor_copy(kpT[pt][:, 128 * ct:128 * (ct + 1)], pst[:])
    for ct in range(CT):
        for pt in range(8):
            pst = ps_t.tile([128, 128], BF, tag="ps_t", name="ps_trq")
            nc.tensor.transpose(pst[:], q3[ct][:, 128 * pt:128 * (pt + 1)],
                                idn_t[:])
            nc.vector.tensor_copy(q3T[pt][:, 128 * ct:128 * (ct + 1)], pst[:])
    _vdw_ct(1)
    _vdw_ct(2)

    # ---------------- F: QK + softmax + M (per head) ----------------
    mst = [big.tile([128, DIM], BF, tag=f"mst{ct}", name=f"mst{ct}")
           for ct in range(CT)]
    att_n = []
    for h in range(HEADS):
        cs = slice(HC * h, HC * (h + 1))
        pa = ps_t.tile([HC, HC], F32, tag="ps_t", name="ps_at")
        for pt in range(8):
            nc.tensor.matmul(pa[:], q3T[pt][:, cs], kpT[pt][:, cs],
                             start=(pt == 0), stop=(pt == 7))
        ae = wrk.tile([HC, HC], BF, tag=f"ae{h % 2}", name=f"ae{h % 2}", bufs=2)
        nc.scalar.activation(ae[:], pa[:], AF.Exp, bias=zero_col[0:HC, :])
        zs = wrk.tile([HC, 1], F32, tag="zs", name="zs")
        nc.vector.tensor_reduce(zs[:], ae[:], axis=mybir.AxisListType.X,
                                op=OP.add)
        zi = wrk.tile([HC, 1], F32, tag="zi", name="zi")
        nc.vector.reciprocal(zi[:], zs[:])
        an = wrk.tile([HC, HC], BF, tag=f"an{h}", name=f"an{h}")
        nc.vector.tensor_scalar_mul(an[:], ae[:], zi[:])
        att_n.append(an)
    for h in range(HEADS):
        pm = ps_t.tile([HC, DIM], F32, tag="ps_t", name="ps_M")
        nc.tensor.matmul(pm[:], att_n[h][:], wpT_t[h][:], start=True, stop=True)
        stg = wrk.tile([HC, DIM], BF, tag=f"stg{h % 2}", name=f"stg{h % 2}")
        nc.vector.tensor_copy(stg[:], pm[:])
        g0 = HC * h
        t0, o0 = divmod(g0, 128)
        n0 = min(128 - o0, HC)
        nc.sync.dma_start(mst[t0][o0:o0 + n0, :], stg[0:n0, :])
        if n0 < HC:
            nc.sync.dma_start(mst[t0 + 1][0:HC - n0, :], stg[n0:HC, :])

    # ---------------- H: out = Mst.T @ v_dw ----------------
    for ob in range(CT):
        for g4 in range(4):
            ps = ps_big.tile([128, 1024], F32, tag="ps", name="ps_h")
            for ctd in range(CT):
                for j in range(2):
                    nc.tensor.matmul(
                        ps[:, 512 * j:512 * (j + 1)],
                        mst[ctd][:, 128 * ob:128 * (ob + 1)],
                        v_dw[ctd][:, 1024 * g4 + 512 * j:1024 * g4 + 512 * (j + 1)],
                        start=(ctd == 0), stop=(ctd == CT - 1))
            ot = osb.tile([128, 1024], BF, tag="osb", name="osb", bufs=3)
            if g4 % 2 == 0:
                nc.scalar.copy(ot[:], ps[:])
            else:
                nc.vector.tensor_copy(ot[:], ps[:])
            eng = nc.sync if g4 % 2 == 0 else nc.scalar
            eng.dma_start(out_d.ap()[128 * ob:128 * (ob + 1),
                                     1024 * g4:1024 * (g4 + 1)], ot[:])
    ctx.close()


# ======================= host-side wrapper =======================

def _prep_shared(w_kv, w_kv_dw, w_q, w_q_dw, w_proj, temperature):
    w_kv = np.asarray(w_kv, np.float32)[:, :, 0, 0]          # [768, 384]
    w_kv_dw = np.asarray(w_kv_dw, np.float32)[:, 0]          # [768, 3, 3]
    w_q = np.asarray(w_q, np.float32)[:, :, 0, 0]            # [384, 384]
    w_q_dw = np.asarray(w_q_dw, np.float32)                  # [384, 384, 3, 3]
    w_proj = np.asarray(w_proj, np.float32)[:, :, 0, 0]      # [384, 384]
    temperature = np.asarray(temperature, np.float32).reshape(HEADS)

    # wkvv: v-half 1x1 weights, [in 384, out 384] transposed, bf16
    wkvv = np.ascontiguousarray(w_kv[DIM:].T).astype(BF_NP)
    # wkv8: k-half fp8 [in 384, ci-major 3 x out 384]; scale K_SCALE
    wk = w_kv[:DIM].T * K_SCALE                              # [in, out]
    wkv8 = np.ascontiguousarray(wk).astype(F8_NP)            # rows = in
    # wq8 similarly, scaled Q_SCALE
    wq8 = np.ascontiguousarray(w_q.T * Q_SCALE).astype(F8_NP)

    w3v = w_kv_dw[DIM:].reshape(DIM, 9)
    w3k = w_kv_dw[:DIM]
    w4k = np.zeros((DIM, 4, 4), np.float32)
    for uy in range(4):
        for ux in range(4):
            acc = np.zeros(DIM, np.float32)
            for dy in range(2):
                for dx in range(2):
                    ky, kx = uy - dy, ux - dx
                    if 0 <= ky < 3 and 0 <= kx < 3:
                        acc += w3k[:, ky, kx]
            w4k[:, uy, ux] = 0.25 * acc * W4K_SCALE
    w4k = w4k.reshape(DIM, 16)

    # diag weights baked on host
    ey = np.eye(128, dtype=np.float32)
    kdiag = np.zeros((128, 3, 8, 2, 128), np.float32)
    for ct in range(3):
        for g in range(8):
            uy, ux = g // 4, g % 4
            for s in range(2):
                u = (uy + 2 * s) * 4 + ux
                kdiag[:, ct, g, s, :] = ey * w4k[128 * ct:128 * (ct + 1),
                                                u][:, None]
    kdiag8 = kdiag.reshape(128, 3 * 8 * 2 * 128).astype(F8_NP)
    vdiag = np.zeros((128, 3, 9, 128), np.float32)
    for ct in range(3):
        for t9 in range(9):
            w = w3v[128 * ct:128 * (ct + 1), t9][:, None]
            vdiag[:, ct, t9, :] = ey * w
    vdiagb = vdiag.reshape(128, 3 * 9 * 128).astype(BF_NP)

    # wqdw8: pairs [(0,1),(3,4),(6,7),(2,5),(8,-)] as [in, grp, co, s, 128]
    wqdwT = np.transpose(w_q_dw, (1, 2, 3, 0)).reshape(DIM, 9, DIM) * Q_SCALE
    pair_taps = [(0, 1), (3, 4), (6, 7), (2, 5), (8, None)]
    wqdw8 = np.zeros((DIM, 5, 3, 2, 128), np.float32)
    for gi, (ta, tb) in enumerate(pair_taps):
        for co in range(3):
            wqdw8[:, gi, co, 0, :] = wqdwT[:, ta, 128 * co:128 * (co + 1)]
            if tb is not None:
                wqdw8[:, gi, co, 1, :] = wqdwT[:, tb, 128 * co:128 * (co + 1)]
    wqdw8 = wqdw8.reshape(DIM, 5 * 3 * 2 * 128).astype(F8_NP)

    wpT = np.ascontiguousarray(w_proj.T).astype(BF_NP)
    temp_col = np.repeat(temperature, HC)[:, None].astype(np.float32)
    idn = np.eye(128, dtype=BF_NP)
    return dict(wkvv=wkvv, wkv8=wkv8, wq8=wq8, wqdw8=wqdw8, kdiag8=kdiag8,
                vdiagb=vdiagb, wpT=wpT, temp_col=temp_col, idn=idn)


_NC_CACHE = {}


def _get_nc(dbg=False):
    key = bool(dbg)
    if key not in _NC_CACHE:
        _NC_CACHE[key] = build_program()
    return _NC_CACHE[key]


def make_in_maps(x, y, shared):
    x = np.asarray(x, np.float32)
    y = np.asarray(y, np.float32)
    B = x.shape[0]
    in_maps = []
    for b in range(B):
        m = dict(shared)
        xr = np.ascontiguousarray(x[b].reshape(DIM, NPIX))
        m["x_bf"] = xr.astype(BF_NP)
        m["x_f8"] = xr.astype(F8_NP)
        m["y_f8"] = np.ascontiguousarray(y[b].reshape(DIM, NPIX2)).astype(F8_NP)
        in_maps.append(m)
    return in_maps


def kernel(x, y, w_kv, w_kv_dw, w_q, w_q_dw, w_proj, temperature):
    nc = _get_nc(dbg=False)
    shared = _prep_shared(w_kv, w_kv_dw, w_q, w_q_dw, w_proj, temperature)
    in_maps = make_in_maps(x, y, shared)
    res = run_bass_kernel_spmd(nc, in_maps, core_ids=list(range(len(in_maps))))
    out = np.stack([np.asarray(r["out"], dtype=np.float32).reshape(DIM, H, H)
                    for r in res.results])
    return out
